# revision 3
# baseline (speedup 1.0000x reference)
"""IoU loss kernel for Trainium2 (8 NeuronCores).

The reference builds a full [N,N] pairwise IoU matrix and takes its
diagonal, so the computation actually needed is elementwise IoU between
pred_boxes[i] and target_boxes[i], followed by mean(1 - iou).

Sharding: row-shard the (diagonal) pairs — 1024 box pairs per core.
Each core DMAs one interleaved [128, 64] f32 tile (8 pred+target box
pairs per partition), computes IoU with 12 fused DVE instructions, and
writes back [128, 1] per-partition partial sums. Host reduces the
8x128 partials and forms the scalar loss.
"""

import numpy as np

import concourse.bass as bass
import concourse.mybir as mybir
from concourse.bass_utils import run_bass_kernel_spmd

N = 8192
NCORES = 8
NLOC = N // NCORES  # 1024 box pairs per core
P = 128             # SBUF partitions
B = NLOC // P       # 8 box pairs per partition
EPS = 1e-07

_NC_CACHE = {}


def _build_nc() -> bass.Bass:
    A = mybir.AluOpType
    f32 = mybir.dt.float32

    nc = bass.Bass()
    # Interleaved per-core input: partition p holds B box pairs, each as
    # 8 floats [p_cx, p_cy, p_w, p_h, t_cx, t_cy, t_w, t_h].
    boxes = nc.declare_dram_parameter("boxes", [P, B * 8], f32, isOutput=False)
    out = nc.declare_dram_parameter("out", [P, 1], f32, isOutput=True)

    with (
        nc.sbuf_tensor([P, B * 8], f32) as bx,
        nc.sbuf_tensor([P, B * 4], f32) as lo,
        nc.sbuf_tensor([P, B * 4], f32) as hi,
        nc.sbuf_tensor([P, B * 2], f32) as ilo,
        nc.sbuf_tensor([P, B * 2], f32) as ihi,
        nc.sbuf_tensor([P, B * 2], f32) as d,
        nc.sbuf_tensor([P, B * 2], f32) as areas,
        nc.sbuf_tensor([P, B], f32) as inter,
        nc.sbuf_tensor([P, B], f32) as s,
        nc.sbuf_tensor([P, B], f32) as denom,
        nc.sbuf_tensor([P, B], f32) as recip,
        nc.sbuf_tensor([P, B], f32) as iou,
        nc.sbuf_tensor([P, 1], f32) as res,
        nc.semaphore("dma_sem") as dma_sem,
        nc.semaphore("v_sem") as v_sem,
        nc.Block() as block,
    ):

        @block.sync
        def _(sync):
            sync.dma_start(out=bx[:], in_=boxes[:]).then_inc(dma_sem, 16)
            sync.wait_ge(v_sem, 12)
            sync.dma_start(out=out[:], in_=res[:]).then_inc(dma_sem, 16)
            sync.wait_ge(dma_sem, 32)

        @block.vector
        def _(vector):
            vector.wait_ge(dma_sem, 16)

            # [P, B, 2(groups: pred/target), 4(fields: cx,cy,w,h)]
            v4 = bx[:].rearrange("p (b g f) -> p b g f", g=2, f=4)
            cxy = v4[:, :, :, 0:2]
            wh = v4[:, :, :, 2:4]
            lo4 = lo[:].rearrange("p (b g f) -> p b g f", g=2, f=2)
            hi4 = hi[:].rearrange("p (b g f) -> p b g f", g=2, f=2)
            ilo3 = ilo[:].rearrange("p (b f) -> p b f", f=2)
            ihi3 = ihi[:].rearrange("p (b f) -> p b f", f=2)
            d3 = d[:].rearrange("p (b f) -> p b f", f=2)
            ar3 = areas[:].rearrange("p (b g) -> p b g", g=2)

            # The DVE pipeline does not interlock same-engine RAW through
            # SBUF; every dependent op must wait for the producer's
            # completion semaphore.
            n_done = 0

            def S(inst):
                nonlocal n_done
                n_done += 1
                inst.then_inc(v_sem, 1)
                vector.wait_ge(v_sem, n_done)

            # corners: lo = cxy - wh/2, hi = cxy + wh/2 (pred and target at once)
            S(vector.scalar_tensor_tensor(lo4, wh, -0.5, cxy, A.mult, A.add))
            S(vector.scalar_tensor_tensor(hi4, wh, 0.5, cxy, A.mult, A.add))
            # intersection box
            S(vector.tensor_tensor(ilo3, lo4[:, :, 0, :], lo4[:, :, 1, :], A.max))
            S(vector.tensor_tensor(ihi3, hi4[:, :, 0, :], hi4[:, :, 1, :], A.min))
            # clipped widths: relu(ihi - ilo)
            S(vector.scalar_tensor_tensor(d3, ilo3, -1.0, ihi3, A.mult, A.add))
            S(vector.tensor_scalar_max(d[:], d[:], 0.0))
            S(vector.tensor_tensor(inter[:], d3[:, :, 0], d3[:, :, 1], A.mult))
            # areas = w*h for pred and target in one op
            S(vector.tensor_tensor(ar3, v4[:, :, :, 2], v4[:, :, :, 3], A.mult))
            # denom = area_p + area_t - inter + EPS
            S(vector.scalar_tensor_tensor(s[:], ar3[:, :, 0], EPS, ar3[:, :, 1], A.add, A.add))
            S(vector.scalar_tensor_tensor(denom[:], inter[:], -1.0, s[:], A.mult, A.add))
            # iou = inter / denom; accumulate per-partition sum into res
            S(vector.reciprocal(recip[:], denom[:]))
            S(vector.scalar_tensor_tensor(
                iou[:], recip[:], 0.0, inter[:], A.bypass, A.mult, accum_out=res[:]
            ))

    return nc


def kernel(pred_boxes: np.ndarray, target_boxes: np.ndarray) -> np.ndarray:
    pred = np.ascontiguousarray(np.asarray(pred_boxes, dtype=np.float32))
    targ = np.ascontiguousarray(np.asarray(target_boxes, dtype=np.float32))
    assert pred.shape == (N, 4) and targ.shape == (N, 4)

    if "nc" not in _NC_CACHE:
        _NC_CACHE["nc"] = _build_nc()
    nc = _NC_CACHE["nc"]

    in_maps = []
    for c in range(NCORES):
        sl = slice(c * NLOC, (c + 1) * NLOC)
        pair = np.concatenate([pred[sl], targ[sl]], axis=1)  # [NLOC, 8]
        in_maps.append({"boxes": np.ascontiguousarray(pair.reshape(P, B * 8))})

    results = run_bass_kernel_spmd(nc, in_maps, core_ids=list(range(NCORES))).results

    total = np.float64(0.0)
    for r in results:
        total += r["out"].astype(np.float64).sum()
    loss = 1.0 - total / N
    return np.asarray(loss, dtype=np.float32)


# revision 5
# speedup vs baseline: 1.4036x; 1.4036x over previous
"""IoU loss kernel for Trainium2 (8 NeuronCores).

The reference builds a full [N,N] pairwise IoU matrix and takes its
diagonal, so the computation actually needed is elementwise IoU between
pred_boxes[i] and target_boxes[i], followed by mean(1 - iou).

Sharding: row-shard the (diagonal) pairs — 1024 box pairs per core.
Each core DMAs one interleaved [128, 64] f32 tile (8 pred+target box
pairs per partition, split across two DMA queues), computes IoU with 11
fused DVE instructions separated by pipeline DRAINs (the DVE does not
interlock same-engine RAW through SBUF), reduces across partitions with
a stream-transpose + reduce, and writes 4 partial sums per core. The
host reduces 8x4 partials into the scalar loss.
"""

import numpy as np

import concourse.bass as bass
import concourse.mybir as mybir
from concourse.bass_utils import run_bass_kernel_spmd

N = 8192
NCORES = 8
NLOC = N // NCORES  # 1024 box pairs per core
P = 128             # SBUF partitions
B = NLOC // P       # 8 box pairs per partition
EPS = 1e-07

_NC_CACHE = {}


def _build_nc() -> bass.Bass:
    A = mybir.AluOpType
    f32 = mybir.dt.float32

    nc = bass.Bass()
    # Interleaved per-core input: partition p holds B box pairs, each as
    # 8 floats [p_cx, p_cy, p_w, p_h, t_cx, t_cy, t_w, t_h].
    boxes = nc.declare_dram_parameter("boxes", [P, B * 8], f32, isOutput=False)
    out = nc.declare_dram_parameter("out", [4, 1], f32, isOutput=True)

    with (
        nc.sbuf_tensor([P, B * 8], f32) as bx,
        nc.sbuf_tensor([P, B * 4], f32) as lo,
        nc.sbuf_tensor([P, B * 4], f32) as hi,
        nc.sbuf_tensor([P, B * 2], f32) as ilo,
        nc.sbuf_tensor([P, B * 2], f32) as ihi,
        nc.sbuf_tensor([P, B * 2], f32) as d,
        nc.sbuf_tensor([P, B * 2], f32) as areas,
        nc.sbuf_tensor([P, B], f32) as inter,
        nc.sbuf_tensor([P, B], f32) as s,
        nc.sbuf_tensor([P, B], f32) as denom,
        nc.sbuf_tensor([P, B], f32) as iou,
        nc.sbuf_tensor([P, 32], f32) as res32,
        nc.sbuf_tensor([P, 32], f32) as tr,
        nc.sbuf_tensor([P, 1], f32) as t2,
        nc.semaphore("dma_sem") as dma_sem,
        nc.semaphore("gdma_sem") as gdma_sem,
        nc.semaphore("v_sem") as v_sem,
        nc.Block() as block,
    ):

        @block.sync
        def _(sync):
            sync.dma_start(out=bx[0:64, :], in_=boxes[0:64, :]).then_inc(dma_sem, 16)
            sync.wait_ge(v_sem, 1)
            # 4 partial sums live in partitions {0,32,64,96} after the
            # transpose+reduce; a 4-descriptor DMA beats the 128-descriptor
            # scatter of a [128,1] store by ~6us.
            sync.dma_start(out=out[:], in_=t2[0:128:32, :]).then_inc(dma_sem, 16)
            sync.wait_ge(dma_sem, 32)

        @block.gpsimd
        def _(gpsimd):
            gpsimd.dma_start(out=bx[64:128, :], in_=boxes[64:128, :]).then_inc(gdma_sem, 16)

        @block.vector
        def _(vector):
            vector.memset(res32[:], 0.0)
            vector.wait_ge(dma_sem, 16)
            vector.wait_ge(gdma_sem, 16)

            # [P, B, 2(groups: pred/target), 4(fields: cx,cy,w,h)]
            v4 = bx[:].rearrange("p (b g f) -> p b g f", g=2, f=4)
            cxy = v4[:, :, :, 0:2]
            wh = v4[:, :, :, 2:4]
            lo4 = lo[:].rearrange("p (b g f) -> p b g f", g=2, f=2)
            hi4 = hi[:].rearrange("p (b g f) -> p b g f", g=2, f=2)
            ilo3 = ilo[:].rearrange("p (b f) -> p b f", f=2)
            ihi3 = ihi[:].rearrange("p (b f) -> p b f", f=2)
            d3 = d[:].rearrange("p (b f) -> p b f", f=2)
            ar3 = areas[:].rearrange("p (b g) -> p b g", g=2)

            # corners: lo = cxy - wh/2, hi = cxy + wh/2 (pred and target at once)
            vector.scalar_tensor_tensor(lo4, wh, -0.5, cxy, A.mult, A.add)
            vector.scalar_tensor_tensor(hi4, wh, 0.5, cxy, A.mult, A.add)
            # areas = w*h for pred and target in one op
            vector.tensor_tensor(ar3, v4[:, :, :, 2], v4[:, :, :, 3], A.mult)
            vector.drain()
            # intersection box
            vector.tensor_tensor(ilo3, lo4[:, :, 0, :], lo4[:, :, 1, :], A.max)
            vector.drain()
            vector.tensor_tensor(ihi3, hi4[:, :, 0, :], hi4[:, :, 1, :], A.min)
            vector.drain()
            # s = area_p + EPS + area_t
            vector.scalar_tensor_tensor(s[:], ar3[:, :, 0], EPS, ar3[:, :, 1], A.add, A.add)
            vector.drain()
            # clipped intersection widths: relu(ihi - ilo)
            vector.scalar_tensor_tensor(d3, ilo3, -1.0, ihi3, A.mult, A.add)
            vector.drain()
            vector.scalar_tensor_tensor(d3, d3, 0.0, d3, A.max, A.bypass)
            vector.drain()
            vector.tensor_tensor(inter[:], d3[:, :, 0], d3[:, :, 1], A.mult)
            vector.drain()
            # denom = s - inter  (EPS already folded into s)
            vector.scalar_tensor_tensor(denom[:], inter[:], -1.0, s[:], A.mult, A.add)
            vector.drain()
            # iou = inter/denom with per-partition sum into res32 col 0
            vector.reciprocal(denom[:], denom[:])
            vector.drain()
            vector.scalar_tensor_tensor(
                iou[:], denom[:], 0.0, inter[:], A.bypass, A.mult,
                accum_out=res32[:, 0:1],
            )
            vector.drain()
            # cross-partition reduce: 32x32 stream transpose moves the 128
            # partials into rows {0,32,64,96}, then reduce along free dim.
            vector.transpose(tr[:], res32[:])
            vector.drain()
            vector.reduce_sum(t2[:], tr[:], axis=mybir.AxisListType.X)
            vector.drain().then_inc(v_sem, 1)

    return nc


def kernel(pred_boxes: np.ndarray, target_boxes: np.ndarray) -> np.ndarray:
    pred = np.ascontiguousarray(np.asarray(pred_boxes, dtype=np.float32))
    targ = np.ascontiguousarray(np.asarray(target_boxes, dtype=np.float32))
    assert pred.shape == (N, 4) and targ.shape == (N, 4)

    if "nc" not in _NC_CACHE:
        _NC_CACHE["nc"] = _build_nc()
    nc = _NC_CACHE["nc"]

    in_maps = []
    for c in range(NCORES):
        sl = slice(c * NLOC, (c + 1) * NLOC)
        pair = np.concatenate([pred[sl], targ[sl]], axis=1)  # [NLOC, 8]
        in_maps.append({"boxes": np.ascontiguousarray(pair.reshape(P, B * 8))})

    results = run_bass_kernel_spmd(nc, in_maps, core_ids=list(range(NCORES))).results

    total = np.float64(0.0)
    for r in results:
        total += r["out"].astype(np.float64).sum()
    loss = 1.0 - total / N
    return np.asarray(loss, dtype=np.float32)


# revision 7
# speedup vs baseline: 1.5466x; 1.1018x over previous
"""IoU loss kernel for Trainium2 (8 NeuronCores).

The reference builds a full [N,N] pairwise IoU matrix and takes its
diagonal, so the computation actually needed is elementwise IoU between
pred_boxes[i] and target_boxes[i], followed by mean(1 - iou).

Sharding: row-shard the (diagonal) pairs — 1024 box pairs per core.
Each core DMAs one interleaved [128, 64] f32 tile (8 pred+target box
pairs per partition, split across two DMA queues), computes IoU with 11
fused DVE instructions separated by pipeline DRAINs (the DVE does not
interlock same-engine RAW through SBUF), reduces across partitions with
a stream-transpose + reduce, and writes 4 partial sums per core. The
host reduces 8x4 partials into the scalar loss.
"""

import numpy as np

import concourse.bass as bass
import concourse.mybir as mybir
from concourse.bass_utils import run_bass_kernel_spmd

N = 8192
NCORES = 8
NLOC = N // NCORES  # 1024 box pairs per core
P = 128             # SBUF partitions
B = NLOC // P       # 8 box pairs per partition
EPS = 1e-07

_NC_CACHE = {}


def _build_nc() -> bass.Bass:
    A = mybir.AluOpType
    f32 = mybir.dt.float32

    nc = bass.Bass()
    # Interleaved per-core input: partition p holds B box pairs, each as
    # 8 floats [p_cx, p_cy, p_w, p_h, t_cx, t_cy, t_w, t_h].
    boxes = nc.declare_dram_parameter("boxes", [P, B * 8], f32, isOutput=False)
    out = nc.declare_dram_parameter("out", [4, 1], f32, isOutput=True)

    with (
        nc.sbuf_tensor([P, B * 8], f32) as bx,
        nc.sbuf_tensor([P, B * 4], f32) as lo,
        nc.sbuf_tensor([P, B * 4], f32) as hi,
        nc.sbuf_tensor([P, B * 2], f32) as ilo,
        nc.sbuf_tensor([P, B * 2], f32) as ihi,
        nc.sbuf_tensor([P, B * 2], f32) as d,
        nc.sbuf_tensor([P, B * 2], f32) as areas,
        nc.sbuf_tensor([P, B], f32) as inter,
        nc.sbuf_tensor([P, B], f32) as s,
        nc.sbuf_tensor([P, B], f32) as denom,
        nc.sbuf_tensor([P, B], f32) as iou,
        nc.sbuf_tensor([P, 32], f32) as res32,
        nc.sbuf_tensor([P, 32], f32) as tr,
        nc.sbuf_tensor([P, 1], f32) as t2,
        nc.semaphore("dma_sem") as dma_sem,
        nc.semaphore("gdma_sem") as gdma_sem,
        nc.semaphore("v_sem") as v_sem,
        nc.Block() as block,
    ):

        @block.sync
        def _(sync):
            # Asymmetric split: the HWDGE (sync) queue issues ~650ns before
            # the SWDGE (gpsimd) queue clears its preamble, so give it more
            # lines; both halves then complete at about the same time.
            sync.dma_start(out=bx[0:88, :], in_=boxes[0:88, :]).then_inc(dma_sem, 16)
            sync.wait_ge(v_sem, 1)
            # 4 partial sums live in partitions {0,32,64,96} after the
            # transpose+reduce; a 4-descriptor DMA beats the 128-descriptor
            # scatter of a [128,1] store by ~6us. No completion wait: the
            # NEFF postamble (~7us) far outlasts this 16B transfer.
            sync.dma_start(out=out[:], in_=t2[0:128:32, :]).then_inc(dma_sem, 16)

        @block.gpsimd
        def _(gpsimd):
            gpsimd.dma_start(out=bx[88:128, :], in_=boxes[88:128, :]).then_inc(gdma_sem, 16)

        @block.vector
        def _(vector):
            vector.memset(res32[:], 0.0)
            vector.wait_ge(dma_sem, 16)
            vector.wait_ge(gdma_sem, 16)

            # [P, B, 2(groups: pred/target), 4(fields: cx,cy,w,h)]
            v4 = bx[:].rearrange("p (b g f) -> p b g f", g=2, f=4)
            cxy = v4[:, :, :, 0:2]
            wh = v4[:, :, :, 2:4]
            lo4 = lo[:].rearrange("p (b g f) -> p b g f", g=2, f=2)
            hi4 = hi[:].rearrange("p (b g f) -> p b g f", g=2, f=2)
            ilo3 = ilo[:].rearrange("p (b f) -> p b f", f=2)
            ihi3 = ihi[:].rearrange("p (b f) -> p b f", f=2)
            d3 = d[:].rearrange("p (b f) -> p b f", f=2)
            ar3 = areas[:].rearrange("p (b g) -> p b g", g=2)

            # The DVE pipeline write-back tail is ~165ns (one DRAIN); a
            # producer->consumer gap of >=2 ops (~350ns) needs no DRAIN,
            # adjacent dependents do.
            # corners: lo = cxy - wh/2, hi = cxy + wh/2 (pred and target at once)
            vector.scalar_tensor_tensor(lo4, wh, -0.5, cxy, A.mult, A.add)
            vector.scalar_tensor_tensor(hi4, wh, 0.5, cxy, A.mult, A.add)
            # areas = w*h for pred and target in one op
            vector.tensor_tensor(ar3, v4[:, :, :, 2], v4[:, :, :, 3], A.mult)
            # intersection box (lo/hi written >=2 ops ago)
            vector.tensor_tensor(ilo3, lo4[:, :, 0, :], lo4[:, :, 1, :], A.max)
            vector.tensor_tensor(ihi3, hi4[:, :, 0, :], hi4[:, :, 1, :], A.min)
            # s = area_p + EPS + area_t (ar written 2 ops ago)
            vector.scalar_tensor_tensor(s[:], ar3[:, :, 0], EPS, ar3[:, :, 1], A.add, A.add)
            vector.drain()
            # clipped intersection widths: relu(ihi - ilo)
            vector.scalar_tensor_tensor(d3, ilo3, -1.0, ihi3, A.mult, A.add)
            vector.drain()
            vector.scalar_tensor_tensor(d3, d3, 0.0, d3, A.max, A.bypass)
            vector.drain()
            vector.tensor_tensor(inter[:], d3[:, :, 0], d3[:, :, 1], A.mult)
            vector.drain()
            # denom = s - inter  (EPS already folded into s)
            vector.scalar_tensor_tensor(denom[:], inter[:], -1.0, s[:], A.mult, A.add)
            vector.drain()
            # iou = inter/denom with per-partition sum into res32 col 0
            vector.reciprocal(denom[:], denom[:])
            vector.drain()
            vector.scalar_tensor_tensor(
                iou[:], denom[:], 0.0, inter[:], A.bypass, A.mult,
                accum_out=res32[:, 0:1],
            )
            vector.drain()
            # cross-partition reduce: 32x32 stream transpose moves the 128
            # partials into rows {0,32,64,96}, then reduce along free dim.
            vector.transpose(tr[:], res32[:])
            vector.drain()
            vector.reduce_sum(t2[:], tr[:], axis=mybir.AxisListType.X)
            vector.drain().then_inc(v_sem, 1)

    return nc


def kernel(pred_boxes: np.ndarray, target_boxes: np.ndarray) -> np.ndarray:
    pred = np.ascontiguousarray(np.asarray(pred_boxes, dtype=np.float32))
    targ = np.ascontiguousarray(np.asarray(target_boxes, dtype=np.float32))
    assert pred.shape == (N, 4) and targ.shape == (N, 4)

    if "nc" not in _NC_CACHE:
        _NC_CACHE["nc"] = _build_nc()
    nc = _NC_CACHE["nc"]

    in_maps = []
    for c in range(NCORES):
        sl = slice(c * NLOC, (c + 1) * NLOC)
        pair = np.concatenate([pred[sl], targ[sl]], axis=1)  # [NLOC, 8]
        in_maps.append({"boxes": np.ascontiguousarray(pair.reshape(P, B * 8))})

    results = run_bass_kernel_spmd(nc, in_maps, core_ids=list(range(NCORES))).results

    total = np.float64(0.0)
    for r in results:
        total += r["out"].astype(np.float64).sum()
    loss = 1.0 - total / N
    return np.asarray(loss, dtype=np.float32)


# revision 11
# speedup vs baseline: 1.5528x; 1.0040x over previous
"""IoU loss kernel for Trainium2 (8 NeuronCores).

The reference builds a full [N,N] pairwise IoU matrix and takes its
diagonal, so the computation actually needed is elementwise IoU between
pred_boxes[i] and target_boxes[i], followed by mean(1 - iou).

Sharding: row-shard the (diagonal) pairs — 1024 box pairs per core.
Each core DMAs one interleaved [128, 64] f32 tile (8 pred+target box
pairs per partition, split across two DMA queues), computes IoU with 11
fused DVE instructions separated by pipeline DRAINs (the DVE does not
interlock same-engine RAW through SBUF), reduces across partitions with
a stream-transpose + reduce, and writes 4 partial sums per core. The
host reduces 8x4 partials into the scalar loss.
"""

import numpy as np

import concourse.bass as bass
import concourse.mybir as mybir
from concourse.bass_utils import run_bass_kernel_spmd

N = 8192
NCORES = 8
NLOC = N // NCORES  # 1024 box pairs per core
P = 128             # SBUF partitions
B = NLOC // P       # 8 box pairs per partition
EPS = 1e-07

_NC_CACHE = {}


def _build_nc() -> bass.Bass:
    A = mybir.AluOpType
    f32 = mybir.dt.float32

    nc = bass.Bass()
    # Interleaved per-core input: partition p holds B box pairs, each as
    # 8 floats [p_cx, p_cy, p_w, p_h, t_cx, t_cy, t_w, t_h].
    boxes = nc.declare_dram_parameter("boxes", [P, B * 8], f32, isOutput=False)
    out = nc.declare_dram_parameter("out", [4, 1], f32, isOutput=True)

    with (
        nc.sbuf_tensor([P, B * 8], f32) as bx,
        nc.sbuf_tensor([P, B * 4], f32) as lo,
        nc.sbuf_tensor([P, B * 4], f32) as hi,
        nc.sbuf_tensor([P, B * 2], f32) as ilo,
        nc.sbuf_tensor([P, B * 2], f32) as ihi,
        nc.sbuf_tensor([P, B * 2], f32) as d,
        nc.sbuf_tensor([P, B * 2], f32) as areas,
        nc.sbuf_tensor([P, B], f32) as inter,
        nc.sbuf_tensor([P, B], f32) as s,
        nc.sbuf_tensor([P, B], f32) as denom,
        nc.sbuf_tensor([P, B], f32) as iou,
        nc.sbuf_tensor([P, 32], f32) as res32,
        nc.sbuf_tensor([P, 32], f32) as tr,
        nc.sbuf_tensor([P, 1], f32) as t2,
        nc.semaphore("dma_sem") as dma_sem,
        nc.semaphore("v_sem") as v_sem,
        nc.Block() as block,
    ):
        # Input split across the two HWDGE queues (sync + scalar); both
        # engines clear their preamble at ~1.1-1.3us, unlike gpsimd's
        # SWDGE path which starts ~800ns later. Both increment one
        # semaphore.

        @block.sync
        def _(sync):
            sync.dma_start(out=bx[0:64, :], in_=boxes[0:64, :]).then_inc(dma_sem, 16)
            sync.wait_ge(v_sem, 1)
            # 4 partial sums live in partitions {0,32,64,96} after the
            # transpose+reduce; a 4-descriptor DMA beats the 128-descriptor
            # scatter of a [128,1] store by ~6us. No completion wait: the
            # NEFF postamble (~7us) far outlasts this 16B transfer.
            sync.dma_start(out=out[:], in_=t2[0:128:32, :]).then_inc(dma_sem, 16)

        @block.scalar
        def _(scalar):
            scalar.dma_start(out=bx[64:128, :], in_=boxes[64:128, :]).then_inc(dma_sem, 16)

        @block.vector
        def _(vector):
            vector.memset(res32[:], 0.0)
            vector.wait_ge(dma_sem, 32)

            # [P, B, 2(groups: pred/target), 4(fields: cx,cy,w,h)]
            v4 = bx[:].rearrange("p (b g f) -> p b g f", g=2, f=4)
            cxy = v4[:, :, :, 0:2]
            wh = v4[:, :, :, 2:4]
            lo4 = lo[:].rearrange("p (b g f) -> p b g f", g=2, f=2)
            hi4 = hi[:].rearrange("p (b g f) -> p b g f", g=2, f=2)
            ilo3 = ilo[:].rearrange("p (b f) -> p b f", f=2)
            ihi3 = ihi[:].rearrange("p (b f) -> p b f", f=2)
            d3 = d[:].rearrange("p (b f) -> p b f", f=2)
            ar3 = areas[:].rearrange("p (b g) -> p b g", g=2)

            # The DVE pipeline write-back tail is ~165ns (one DRAIN); a
            # producer->consumer gap of >=2 ops (~350ns) needs no DRAIN,
            # adjacent dependents do.
            # corners: lo = cxy - wh/2, hi = cxy + wh/2 (pred and target at once)
            vector.scalar_tensor_tensor(lo4, wh, -0.5, cxy, A.mult, A.add)
            vector.scalar_tensor_tensor(hi4, wh, 0.5, cxy, A.mult, A.add)
            # areas = w*h for pred and target in one op
            vector.tensor_tensor(ar3, v4[:, :, :, 2], v4[:, :, :, 3], A.mult)
            # intersection box (lo/hi written >=2 ops ago)
            vector.tensor_tensor(ilo3, lo4[:, :, 0, :], lo4[:, :, 1, :], A.max)
            vector.tensor_tensor(ihi3, hi4[:, :, 0, :], hi4[:, :, 1, :], A.min)
            # s = area_p + EPS + area_t (ar written 2 ops ago)
            vector.scalar_tensor_tensor(s[:], ar3[:, :, 0], EPS, ar3[:, :, 1], A.add, A.add)
            vector.drain()
            # clipped intersection widths: relu(ihi - ilo)
            vector.scalar_tensor_tensor(d3, ilo3, -1.0, ihi3, A.mult, A.add)
            vector.drain()
            vector.scalar_tensor_tensor(d3, d3, 0.0, d3, A.max, A.bypass)
            vector.drain()
            vector.tensor_tensor(inter[:], d3[:, :, 0], d3[:, :, 1], A.mult)
            vector.drain()
            # denom = s - inter  (EPS already folded into s)
            vector.scalar_tensor_tensor(denom[:], inter[:], -1.0, s[:], A.mult, A.add)
            vector.drain()
            # iou = inter/denom with per-partition sum into res32 col 0
            vector.reciprocal(denom[:], denom[:])
            vector.drain()
            vector.scalar_tensor_tensor(
                iou[:], denom[:], 0.0, inter[:], A.bypass, A.mult,
                accum_out=res32[:, 0:1],
            )
            vector.drain()
            # cross-partition reduce: 32x32 stream transpose moves the 128
            # partials into rows {0,32,64,96}, then reduce along free dim.
            vector.transpose(tr[:], res32[:])
            vector.drain()
            vector.reduce_sum(t2[:], tr[:], axis=mybir.AxisListType.X)
            vector.drain().then_inc(v_sem, 1)

    return nc


def kernel(pred_boxes: np.ndarray, target_boxes: np.ndarray) -> np.ndarray:
    pred = np.ascontiguousarray(np.asarray(pred_boxes, dtype=np.float32))
    targ = np.ascontiguousarray(np.asarray(target_boxes, dtype=np.float32))
    assert pred.shape == (N, 4) and targ.shape == (N, 4)

    if "nc" not in _NC_CACHE:
        _NC_CACHE["nc"] = _build_nc()
    nc = _NC_CACHE["nc"]

    in_maps = []
    for c in range(NCORES):
        sl = slice(c * NLOC, (c + 1) * NLOC)
        pair = np.concatenate([pred[sl], targ[sl]], axis=1)  # [NLOC, 8]
        in_maps.append({"boxes": np.ascontiguousarray(pair.reshape(P, B * 8))})

    results = run_bass_kernel_spmd(nc, in_maps, core_ids=list(range(NCORES))).results

    total = np.float64(0.0)
    for r in results:
        total += r["out"].astype(np.float64).sum()
    loss = 1.0 - total / N
    return np.asarray(loss, dtype=np.float32)
